# revision 2
# baseline (speedup 1.0000x reference)
"""Trainium2 Bass kernel for nn_Experts (64-expert batched LSTM cell).

Math (reference):
    gates[n,b,:] = x[b,:] @ W_ih[n].T + h0[b,:] @ W_hh[n].T + b_ih[n] + b_hh[n]
    i,f,g,o = split(gates, 4);  c' = sig(f)*c0 + sig(i)*tanh(g);  h = sig(o)*tanh(c')
    out[b, n*H+h] = h[n,b,h]            # [B, N*H] = [4096, 4096]

Distribution: expert-parallel over 8 cores; core c owns experts 8c..8c+7 and
produces the contiguous output column block out[:, c*512:(c+1)*512]. All
transposes / weight reordering / bias folding are done host-side in numpy so
the device kernel is pure matmul + activation + elementwise.

Per-core device layout (E=8 local experts, GW=E*H=512), matmul operands bf16,
activations bf16, output bf16 (host converts to fp32 and applies the final
x0.5):
  - xT    [128, 4096]  x transposed      (stationary operands for PE)
  - h0T1  [65, 4096]   h0 transposed + ones row (bias trick)
  - wx    [128, 2048]  W_ih reordered: cols = gate-type-major [i|f|o|g] x E x H
  - wh1   [65, 2048]   W_hh reordered + last row = (b_ih+b_hh) reordered
  - c0s   [128, 32, 64] c0 tiled (broadcast across experts on-chip)
  (wx/wh1 i,f,o columns and bias pre-scaled by 0.5 host-side, exact in bf16)

Engine assignment per batch tile bt (32 tiles of 128 rows), chosen from the
TimelineSim cost table (per [128,512] op: DVE tensor_scalar 194ns / tensor_
tensor 327ns / STT 594ns; ACT 0.833ns/el + ~185ns fixed; Pool TT add 1111ns):
  PE  : psum[128,2048] = xT_t.T @ wx + h0T1_t.T @ wh1      (8 matmuls)
  ACT : ONE tanh over all 2048 gate cols -> sact=[Ti|Tf|To|Tg]   (the 0.5
        pre-scale makes sig(x) = (tanh(x/2)+1)/2)                 1892ns
  DVE : Fs=Tf+1 (194), m2=Fs*c0bc (327), Is=Ti+1 (194),
        m1=Is*Tg (327), Os=To+1 (194)                             1563ns/tile
  Pool: c2q[bt%4] = m1+m2   (= 2c')                               1111ns
  ACT : per QUAD of tiles: tcq = tanh(0.5*c2q) over [128,4,512]   1892ns/quad
  DVE : h2 = Os*tcq  (= 2h, bf16)                                 327ns
  DMA : one 0.5 MB output DMA per quad
ACT is the bottleneck engine at ~75.7us busy; the tail (tcq+h2) runs 4-7
tiles behind the head so ACT never stalls on the Pool/DVE c2 chain.
"""

import numpy as np

import concourse.bass as bass
import concourse.mybir as mybir
from concourse import bacc
from concourse.bass_utils import run_bass_kernel_spmd
from concourse.tile import TileContext

B, D, H, N = 4096, 128, 64, 64
NCORES = 8
EPC = N // NCORES          # experts per core
GW = EPC * H               # 512: width of one gate-type group
FW = 4 * GW                # 2048: full gates free width per batch tile
BT = B // 128              # 32 batch tiles
F32 = mybir.dt.float32
BF16 = mybir.dt.bfloat16

_GATE_ORDER = [0, 1, 3, 2]  # reorder i,f,g,o -> i,f,o,g (sig funcs contiguous)

AF = mybir.ActivationFunctionType
ALU = mybir.AluOpType


def _build_bass() -> bass.Bass:
    nc = bacc.Bacc(None, target_bir_lowering=False, debug=False)
    xT_d = nc.dram_tensor("xT", [D, B], BF16, kind="ExternalInput")
    h0T1_d = nc.dram_tensor("h0T1", [H + 1, B], BF16, kind="ExternalInput")
    c0_d = nc.dram_tensor("c0", [B, H], BF16, kind="ExternalInput")
    wx_d = nc.dram_tensor("wx", [D, FW], BF16, kind="ExternalInput")
    wh1_d = nc.dram_tensor("wh1", [H + 1, FW], BF16, kind="ExternalInput")
    out_d = nc.dram_tensor("out", [B, GW], BF16, kind="ExternalOutput")

    with TileContext(nc) as tc:
        with (
            tc.tile_pool(name="const", bufs=1) as const_pool,
            tc.tile_pool(name="work", bufs=3) as work,
            tc.tile_pool(name="ostage", bufs=2) as ostage,
            tc.tile_pool(name="psum", bufs=2, space="PSUM") as psum_pool,
        ):
            # Chunked const loads, ordered so tile 0 can start almost
            # immediately (needs xT chunk 0 + wx/wh1 i|f|o cols).
            xT = const_pool.tile([D, B], BF16)
            h0T1 = const_pool.tile([H + 1, B], BF16)
            c0sb = const_pool.tile([128, BT, H], BF16)
            c0_v = c0_d.ap().rearrange("(u p) c -> p u c", p=128)
            NCH = 8
            CW = B // NCH
            UCH = BT // NCH
            nc.sync.dma_start(out=xT[:, 0:CW], in_=xT_d[:, 0:CW])
            wx = const_pool.tile([D, FW], BF16)
            wh1 = const_pool.tile([H + 1, FW], BF16)
            IFO = 3 * GW
            nc.sync.dma_start(out=wx[:, 0:IFO], in_=wx_d[:, 0:IFO])
            nc.sync.dma_start(out=wh1[:, 0:IFO], in_=wh1_d[:, 0:IFO])
            nc.sync.dma_start(out=h0T1[:, 0:CW], in_=h0T1_d[:, 0:CW])
            nc.sync.dma_start(out=wx[:, IFO:FW], in_=wx_d[:, IFO:FW])
            nc.sync.dma_start(out=wh1[:, IFO:FW], in_=wh1_d[:, IFO:FW])
            nc.sync.dma_start(out=c0sb[:, 0:UCH], in_=c0_v[:, 0:UCH])
            for k in range(1, NCH):
                ksl = bass.ts(k, CW)
                nc.sync.dma_start(out=xT[:, ksl], in_=xT_d[:, ksl])
                nc.sync.dma_start(out=h0T1[:, ksl], in_=h0T1_d[:, ksl])
                usl = bass.ts(k, UCH)
                nc.sync.dma_start(out=c0sb[:, usl], in_=c0_v[:, usl])

            QN = 4                       # tiles per c'-tanh / output quad
            osd = {}                     # bt -> Os tile (To+1)
            c2qd = {}                    # q -> c2 quad tile

            def head(bt):
                rows = bass.ts(bt, 128)
                psum = psum_pool.tile([128, FW], F32, name=f"ps{bt}", tag="psum")
                # Tile 0: put the g-gate (j=3) matmuls last so the tanh over
                # i|f|o never waits on the second wx/wh1 DMA chunk.
                xjs = [0, 1, 2] if bt == 0 else [0, 1, 2, 3]
                for j in xjs:
                    cols = bass.ts(j, GW)
                    nc.tensor.matmul(psum[:, cols], xT[:, rows], wx[:, cols],
                                     start=True, stop=False)
                for j in xjs:
                    cols = bass.ts(j, GW)
                    nc.tensor.matmul(psum[:, cols], h0T1[:, rows], wh1[:, cols],
                                     start=False, stop=True)
                if len(xjs) == 3:
                    cols = bass.ts(3, GW)
                    nc.tensor.matmul(psum[:, cols], xT[:, rows], wx[:, cols],
                                     start=True, stop=False)
                    nc.tensor.matmul(psum[:, cols], h0T1[:, rows], wh1[:, cols],
                                     start=False, stop=True)

                # sact = [Ti | Tf | To | Tg]: one tanh over ALL gates
                # (i,f,o pre-scaled x0.5 host-side; sig = (T+1)/2).
                sact = work.tile([128, FW], BF16, name=f"sa{bt}", tag="sact")
                if bt == 0:
                    nc.scalar.activation(sact[:, 0:IFO], psum[:, 0:IFO], AF.Tanh)
                    nc.scalar.activation(sact[:, IFO:FW], psum[:, IFO:FW], AF.Tanh)
                else:
                    nc.scalar.activation(sact, psum, AF.Tanh)

                # c2 = 2*c' = (Tf+1)*c0 + (Ti+1)*Tg. Shifts via 4x-mode
                # tensor_scalar adds; products via 2x-mode tensor_tensor;
                # the final add on Pool (off both ACT's and DVE's path).
                c0bc = c0sb[:, bt].unsqueeze(1).broadcast_to([128, EPC, H])
                fs = work.tile([128, GW], BF16, name=f"fs{bt}", tag="fs")
                nc.vector.tensor_scalar_add(fs, sact[:, GW:2 * GW], 1.0)
                m2 = work.tile([128, EPC, H], BF16, name=f"m2{bt}", tag="m2")
                nc.vector.tensor_tensor(
                    m2, fs.rearrange("p (e h) -> p e h", e=EPC), c0bc, ALU.mult)
                is_ = work.tile([128, GW], BF16, name=f"is{bt}", tag="is")
                nc.vector.tensor_scalar_add(is_, sact[:, 0:GW], 1.0)
                m1 = work.tile([128, GW], BF16, name=f"m1{bt}", tag="m1")
                nc.vector.tensor_tensor(m1, is_, sact[:, 3 * GW:FW], ALU.mult)
                if bt % QN == 0:
                    c2qd[bt // QN] = work.tile([128, QN, GW], BF16,
                                               name=f"c2q{bt // QN}", tag="c2q")
                nc.gpsimd.tensor_add(
                    c2qd[bt // QN][:, bt % QN],
                    m1, m2.rearrange("p e h -> p (e h)"))
                os_ = work.tile([128, GW], BF16, name=f"os{bt}", tag="os",
                                bufs=10)
                nc.vector.tensor_scalar_add(os_, sact[:, 2 * GW:3 * GW], 1.0)
                osd[bt] = os_

            out_v = out_d.ap().rearrange("(u p) c -> p u c", p=128)

            def tail(q):
                # tcq = tanh(c') for 4 tiles at once (ACT input scale halves
                # c2); h2 = (To+1)*tanh(c') = 2h, stored bf16 -> one 0.5 MB
                # DMA per quad. Host applies the final x0.5 in fp32.
                tcq = work.tile([128, QN, GW], BF16, name=f"tc{q}", tag="tcq",
                                bufs=2)
                nc.scalar.activation(tcq, c2qd.pop(q), AF.Tanh, scale=0.5)
                hs = ostage.tile([128, QN, GW], BF16, name=f"hs{q}", tag="hs")
                for j in range(QN):
                    nc.vector.tensor_tensor(hs[:, j], osd.pop(q * QN + j),
                                            tcq[:, j], ALU.mult)
                nc.sync.dma_start(out=out_v[:, q * QN:(q + 1) * QN], in_=hs)

            for bt in range(BT):
                head(bt)
                if bt % QN == 3 and bt >= 2 * QN - 1:
                    tail(bt // QN - 1)
            tail(BT // QN - 1)

    nc.compile()
    return nc


def _prep_in_maps(x, h0, c0, W_ih, W_hh, b_ih, b_hh):
    import ml_dtypes

    BF = ml_dtypes.bfloat16
    x = np.asarray(x, np.float32)
    h0 = np.asarray(h0, np.float32)
    c0 = np.asarray(c0, np.float32)
    W_ih = np.asarray(W_ih, np.float32)
    W_hh = np.asarray(W_hh, np.float32)
    b_ih = np.asarray(b_ih, np.float32)
    b_hh = np.asarray(b_hh, np.float32)

    xT = np.ascontiguousarray(x.T).astype(BF)                         # [128, B]
    h0T1 = np.concatenate([h0.T, np.ones((1, B), np.float32)], 0).astype(BF)
    c0b = np.ascontiguousarray(c0).astype(BF)                         # [B, 64]

    Wg = W_ih.reshape(N, 4, H, D)[:, _GATE_ORDER]                     # [n,t,h,d]
    Hg = W_hh.reshape(N, 4, H, H)[:, _GATE_ORDER]                     # [n,t,h,k]
    bg = (b_ih + b_hh).reshape(N, 4, H)[:, _GATE_ORDER]               # [n,t,h]

    in_maps = []
    for c in range(NCORES):
        sl = slice(c * EPC, (c + 1) * EPC)
        wx = Wg[sl].transpose(3, 1, 0, 2).reshape(D, FW).copy()       # [d, t*e*h]
        wh = Hg[sl].transpose(3, 1, 0, 2).reshape(H, FW)
        bias = bg[sl].transpose(1, 0, 2).reshape(1, FW)
        wh1 = np.concatenate([wh, bias], 0)                           # [65, 2048]
        # Pre-scale i,f,o gate columns (incl bias row) by 0.5 — exact in
        # bf16 — so ONE tanh over all gates yields sig(x) = (tanh(x/2)+1)/2.
        wx[:, 0:3 * GW] *= 0.5
        wh1[:, 0:3 * GW] *= 0.5
        in_maps.append({
            "xT": xT,
            "h0T1": h0T1,
            "c0": c0b,
            "wx": np.ascontiguousarray(wx).astype(BF),
            "wh1": np.ascontiguousarray(wh1).astype(BF),
        })
    return in_maps


_NC_CACHE = {}


def _run(in_maps, **kwargs):
    # Cache the built module: repeated kernel() calls then reuse both the
    # Tile-scheduled program and (via the stable nc object) the compiled
    # executable instead of rebuilding/recompiling each time.
    nc = _NC_CACHE.get("nc")
    if nc is None:
        nc = _NC_CACHE["nc"] = _build_bass()
    return run_bass_kernel_spmd(nc, in_maps, list(range(NCORES)), **kwargs)


def kernel(x, h0, c0, W_ih, W_hh, b_ih, b_hh):
    in_maps = _prep_in_maps(x, h0, c0, W_ih, W_hh, b_ih, b_hh)
    res = _run(in_maps)
    # Device output is 2h in bf16; the final x0.5 and fp32 cast are host-side.
    out = np.concatenate(
        [np.asarray(res.results[c]["out"]) for c in range(NCORES)], axis=1
    ).astype(np.float32) * 0.5
    return out, out, out
